# revision 13
# baseline (speedup 1.0000x reference)
"""Causal self-attention (B=4, T=2048, C=1024, H=16, D=64) on 8 TRN2 cores.

Sharding: core c = (batch b = c//2, head-group hg = c%2).  Each core computes
Q/K/V projections for its 8 heads over its batch, causal flash-style
attention in transposed (S^T) layout, then an AllToAll redistributes the
attention output yT from feature-sharded to time-sharded so every core runs
the full out-projection for one t-window of 256 across all 4 batches.

Math layout notes:
  - All matmuls run in float32r (TF32-like, 1 cycle/row for N>=256).
  - Scores are computed transposed: S^T[tk, tq] = (kT chunk)^T-matmul so the
    exp'd probabilities P^T feed the AV matmul directly as the moving operand
    (no P transposes).  K=64 contraction, two heads row-packed per PE pass.
  - Causal masking adds a -1e6 mask into PSUM via an identity matmul before
    the score matmul accumulates on top; exp then underflows masked lanes to 0.
  - V is stored [tk, head, 65] with a baked ones-column so a single M=65 AV
    matmul accumulates both y^T and the softmax denominator (row 64).
"""
import os
import numpy as np

import concourse.bass as bass
import concourse.mybir as mybir
import concourse.tile as tile
from concourse import bacc
from concourse.bass_utils import run_bass_kernel_spmd

B, T, C, H, D = 4, 2048, 1024, 16, 64
NCORES = 8
HPC = H // 2          # heads per core = 8
FPC = HPC * D         # features per core = 512
NKC = C // 128        # contraction chunks = 8
NTQ = T // 512        # tq groups of 512 = 4
NTK = T // 128        # tk chunks of 128 = 16
TW = T // NCORES      # t-window per core after A2A = 256
SCALE = 1.0 / float(np.sqrt(D))
MASK_NEG = -1.0e6

F32 = mybir.dt.float32
F32R = mybir.dt.float32r

_CACHE = {}


def build_nc():
    if "nc" in _CACHE:
        return _CACHE["nc"]
    nc = bacc.Bacc("TRN2", target_bir_lowering=False, debug=False)

    xT_d = nc.dram_tensor("xT", [C, T], F32, kind="ExternalInput")
    wq_d = nc.dram_tensor("wqT", [C, FPC], F32, kind="ExternalInput")
    wk_d = nc.dram_tensor("wkT", [C, FPC], F32, kind="ExternalInput")
    wv_d = nc.dram_tensor("wvT", [C, FPC], F32, kind="ExternalInput")
    wo_d = nc.dram_tensor("woT", [C, C], F32, kind="ExternalInput")
    mask_d = nc.dram_tensor("masks", [128, 4, 512], F32, kind="ExternalInput")
    id_d = nc.dram_tensor("ident", [128, 128], F32, kind="ExternalInput")
    ones_d = nc.dram_tensor("ones_in", [128, 128], F32, kind="ExternalInput")
    out_d = nc.dram_tensor("outT", [B, C, TW], F32, kind="ExternalOutput")
    debug = bool(os.environ.get("KERNEL_DEBUG"))
    if debug:
        dbg_q = nc.dram_tensor("dbg_q", [128, HPC // 2, T], F32, kind="ExternalOutput")
        dbg_k = nc.dram_tensor("dbg_k", [128, HPC // 2, T], F32, kind="ExternalOutput")
        dbg_v = nc.dram_tensor("dbg_v", [128, NTK, HPC, D + 1], F32, kind="ExternalOutput")
        dbg_ys = nc.dram_tensor("dbg_ys", [NCORES * FPC, TW], F32, kind="ExternalOutput")
        dbg_yr = nc.dram_tensor("dbg_yr", [NCORES * FPC, TW], F32, kind="ExternalOutput")
        dbg_yps = nc.dram_tensor("dbg_yps", [D + 1, 512], F32, kind="ExternalOutput")
        dbg_bca = nc.dram_tensor("dbg_bca", [D, 512], F32, kind="ExternalOutput")
        dbg_ysb = nc.dram_tensor("dbg_ysb", [D, 512], F32, kind="ExternalOutput")
        dbg_yn = nc.dram_tensor("dbg_yn", [D, 512], F32, kind="ExternalOutput")

    with tile.TileContext(nc, num_cores=NCORES) as tc:
        with tc.tile_pool(name="persist", bufs=1) as persist, \
             tc.tile_pool(name="consts", bufs=1) as consts, \
             tc.tile_pool(name="dram", bufs=1, space="DRAM") as dram:

            qT = persist.tile([128, HPC // 2, T], F32R)     # [2 heads x 64 d, hp, t]
            kT = persist.tile([128, HPC // 2, T], F32R)
            v = persist.tile([128, NTK, HPC, D + 1], F32R)  # [tk%128, tkc, h, d+ones]
            ident = consts.tile([128, 128], F32R)
            ones_row = consts.tile([1, 64], F32R)
            y_send = dram.tile([NCORES * FPC, TW], F32)
            y_recv = dram.tile([NCORES * FPC, TW], F32)

            nc.sync.dma_start(out=ident, in_=id_d[:].bitcast(F32R))
            nc.sync.dma_start(out=ones_row, in_=ones_d[0:1, 0:64].bitcast(F32R))
            # bake the ones column of v: v[:, tkc, h, 64] = 1.0
            nc.sync.dma_start(
                out=v[:, :, :, D:D + 1],
                in_=ones_d[:].rearrange("p (c h o) -> p c h o", h=HPC, o=1).bitcast(F32R),
            )

            # ---------------- Phase 1: Q/K/V projections ----------------
            with tc.tile_pool(name="weights", bufs=1) as wpool, \
                 tc.tile_pool(name="xs", bufs=2) as xpool, \
                 tc.tile_pool(name="ps1", bufs=6, space="PSUM") as ps1:
                wq = wpool.tile([128, NKC, FPC], F32R)
                wk = wpool.tile([128, NKC, FPC], F32R)
                wv = wpool.tile([128, NKC, FPC], F32R)
                nc.sync.dma_start(out=wq, in_=wq_d[:].rearrange("(c p) f -> p c f", p=128).bitcast(F32R))
                nc.sync.dma_start(out=wk, in_=wk_d[:].rearrange("(c p) f -> p c f", p=128).bitcast(F32R))
                nc.sync.dma_start(out=wv, in_=wv_d[:].rearrange("(c p) f -> p c f", p=128).bitcast(F32R))

                for tg in range(NTQ):
                    xs = xpool.tile([128, NKC, 512], F32R)
                    nc.sync.dma_start(
                        out=xs,
                        in_=xT_d[:, tg * 512:(tg + 1) * 512]
                        .rearrange("(c p) t -> p c t", p=128).bitcast(F32R),
                    )
                    for hp in range(HPC // 2):
                        for which, wt, dst in (("q", wq, qT), ("k", wk, kT)):
                            ps = ps1.tile([128, 512], F32, tag="p1", name=f"ps_{which}{hp}_{tg}")
                            for kc in range(NKC):
                                nc.tensor.matmul(
                                    ps,
                                    wt[:, kc, hp * 128:(hp + 1) * 128],
                                    xs[:, kc, :],
                                    start=(kc == 0), stop=(kc == NKC - 1),
                                )
                            nc.vector.tensor_copy(dst[:, hp, tg * 512:(tg + 1) * 512], ps)
                    for sub in range(4):
                        tkc = tg * 4 + sub
                        ps = ps1.tile([128, FPC], F32, tag="p1", name=f"ps_v{tkc}")
                        for kc in range(NKC):
                            nc.tensor.matmul(
                                ps,
                                xs[:, kc, sub * 128:(sub + 1) * 128],
                                wv[:, kc, :],
                                start=(kc == 0), stop=(kc == NKC - 1),
                            )
                        nc.vector.tensor_copy(
                            v[:, tkc, :, 0:D],
                            ps[:].rearrange("p (h d) -> p h d", h=HPC),
                        )

            # ---------------- Phase 2: causal attention ----------------
            with tc.tile_pool(name="attn", bufs=1) as apool, \
                 tc.tile_pool(name="pt", bufs=4) as ptpool, \
                 tc.tile_pool(name="ynorm", bufs=2) as ypool, \
                 tc.tile_pool(name="ps_s", bufs=2, space="PSUM") as ps_s, \
                 tc.tile_pool(name="ps_y", bufs=1, space="PSUM") as ps_y, \
                 tc.tile_pool(name="ps_b", bufs=2, space="PSUM") as ps_b:
                masks = apool.tile([128, 4, 512], F32R)
                nc.sync.dma_start(out=masks, in_=mask_d[:].bitcast(F32R))

                for hp in range(HPC // 2):
                    for j in range(NTQ):
                        y_ps = [ps_y.tile([D + 1, 512], F32, tag=f"y{w}", name=f"yps{w}_{hp}_{j}")
                                for w in range(2)]
                        n_i = 4 * j + 4
                        for i in range(n_i):
                            m = i - 4 * j
                            s_ps = [ps_s.tile([128, 512], F32, tag=f"s{w}", name=f"sps{w}_{hp}_{j}_{i}")
                                    for w in range(2)]
                            for w in range(2):
                                if m >= 0:
                                    nc.tensor.matmul(s_ps[w], ident, masks[:, m, :],
                                                     start=True, stop=False)
                                nc.tensor.matmul(
                                    s_ps[w],
                                    kT[w * D:(w + 1) * D, hp, i * 128:(i + 1) * 128],
                                    qT[w * D:(w + 1) * D, hp, j * 512:(j + 1) * 512],
                                    start=(m < 0), stop=True,
                                )
                            for w in range(2):
                                p_t = ptpool.tile([128, 512], F32R, tag=f"p{w}", name=f"pt{w}_{hp}_{j}_{i}")
                                nc.scalar.activation(p_t, s_ps[w],
                                                     mybir.ActivationFunctionType.Exp,
                                                     scale=SCALE)
                                nc.tensor.matmul(y_ps[w], v[:, i, 2 * hp + w, :], p_t,
                                                 start=(i == 0), stop=(i == n_i - 1))
                        for w in range(2):
                            recip = ypool.tile([1, 512], F32R, tag=f"r{w}", name=f"rc{w}_{hp}_{j}")
                            with nc.allow_low_precision(reason="f32r recip feeds f32r bcast matmul"):
                                nc.vector.reciprocal(recip, y_ps[w][D:D + 1, :])
                            y_sb = ypool.tile([D, 512], F32, tag=f"ysb{w}", name=f"ysb{w}_{hp}_{j}")
                            nc.vector.tensor_copy(y_sb, y_ps[w][0:D, :])
                            bca = ps_b.tile([D, 512], F32, tag="b", name=f"bca{w}_{hp}_{j}")
                            nc.tensor.matmul(bca, ones_row, recip, start=True, stop=True)
                            yn = ypool.tile([D, 512], F32, tag=f"yn{w}", name=f"yn{w}_{hp}_{j}")
                            nc.vector.tensor_mul(yn, y_sb, bca)
                            if debug and hp == 0 and j == 0 and w == 0:
                                dbg_t = ypool.tile([D + 1, 512], F32, tag="dbg", name="dbg_evict")
                                nc.vector.tensor_copy(dbg_t, y_ps[w])
                                nc.sync.dma_start(out=dbg_yps[:], in_=dbg_t)
                                dbg_t2 = ypool.tile([D, 512], F32, tag="dbg2", name="dbg_evict2")
                                nc.vector.tensor_copy(dbg_t2, bca)
                                nc.sync.dma_start(out=dbg_bca[:], in_=dbg_t2)
                                nc.sync.dma_start(out=dbg_ysb[:], in_=y_sb)
                                nc.sync.dma_start(out=dbg_yn[:], in_=yn)
                            # rows (hp*128 + w*64 .. +64) of this core's yT block,
                            # split across the two 256-wide A2A shards 2j, 2j+1
                            for s in range(2):
                                nc.sync.dma_start(
                                    out=y_send[:]
                                    .rearrange("(s r) t -> s r t", r=FPC)
                                    [2 * j + s, hp * 128 + w * D: hp * 128 + (w + 1) * D, :],
                                    in_=yn[:, s * TW:(s + 1) * TW],
                                )

            if debug:
                nc.sync.dma_start(out=dbg_q[:], in_=qT[:].bitcast(F32))
                nc.sync.dma_start(out=dbg_k[:], in_=kT[:].bitcast(F32))
                nc.sync.dma_start(out=dbg_v[:], in_=v[:].bitcast(F32))
                nc.sync.dma_start(out=dbg_ys[:], in_=y_send[:])

            # ---------------- Phase 3: AllToAll ----------------
            tc.strict_bb_all_engine_barrier()
            nc.gpsimd.collective_compute(
                "AllToAll",
                mybir.AluOpType.bypass,
                replica_groups=[list(range(NCORES))],
                ins=[y_send.opt()],
                outs=[y_recv.opt()],
            )
            tc.strict_bb_all_engine_barrier()

            if debug:
                nc.sync.dma_start(out=dbg_yr[:], in_=y_recv[:])

            # ---------------- Phase 4: out-projection ----------------
            with tc.tile_pool(name="wo", bufs=1) as wopool, \
                 tc.tile_pool(name="yb", bufs=2) as ybpool, \
                 tc.tile_pool(name="oev", bufs=3) as oevpool, \
                 tc.tile_pool(name="ps4", bufs=6, space="PSUM") as ps4:
                wo = wopool.tile([128, NKC, C], F32R)
                nc.sync.dma_start(out=wo, in_=wo_d[:].rearrange("(c p) o -> p c o", p=128).bitcast(F32R))
                for bb in range(B):
                    yb = ybpool.tile([128, NKC, TW], F32R)
                    nc.sync.dma_start(
                        out=yb,
                        in_=y_recv[:]
                        .rearrange("(g p) t -> p g t", p=128)[:, bb * NKC:(bb + 1) * NKC, :]
                        .bitcast(F32R),
                    )
                    for oc in range(NKC):
                        ps = ps4.tile([128, TW], F32, tag="p4", name=f"ps4_{bb}_{oc}")
                        for fc in range(NKC):
                            nc.tensor.matmul(
                                ps,
                                wo[:, fc, oc * 128:(oc + 1) * 128],
                                yb[:, fc, :],
                                start=(fc == 0), stop=(fc == NKC - 1),
                            )
                        oe = oevpool.tile([128, TW], F32, tag="oe", name=f"oe_{bb}_{oc}")
                        nc.vector.tensor_copy(oe, ps)
                        nc.sync.dma_start(out=out_d[bb, oc * 128:(oc + 1) * 128, :], in_=oe)

    nc.compile()
    _CACHE["nc"] = nc
    return nc


def host_constants():
    masks = np.zeros((128, 4, 512), np.float32)
    q_idx = np.arange(512)[None, :]
    p_idx = np.arange(128)[:, None]
    for m in range(4):
        masks[:, m, :] = np.where(q_idx >= p_idx + 128 * m, 0.0, MASK_NEG)
    ident = np.eye(128, dtype=np.float32)
    ones_in = np.ones((128, 128), np.float32)
    return masks, ident, ones_in


def prepare_in_maps(x, Wq, Wk, Wv, Wo):
    masks, ident, ones_in = host_constants()
    woT = np.ascontiguousarray(Wo.T.astype(np.float32, copy=False))
    wqT_f = np.ascontiguousarray(Wq.T.astype(np.float32, copy=False))
    wkT_f = np.ascontiguousarray(Wk.T.astype(np.float32, copy=False))
    wvT_f = np.ascontiguousarray(Wv.T.astype(np.float32, copy=False))
    in_maps = []
    for c in range(NCORES):
        b, hg = c // 2, c % 2
        fs = slice(hg * FPC, (hg + 1) * FPC)
        in_maps.append({
            "xT": np.ascontiguousarray(x[b].T),
            "wqT": np.ascontiguousarray(wqT_f[:, fs]),
            "wkT": np.ascontiguousarray(wkT_f[:, fs]),
            "wvT": np.ascontiguousarray(wvT_f[:, fs]),
            "woT": woT,
            "masks": masks,
            "ident": ident,
            "ones_in": ones_in,
        })
    return in_maps


def assemble(results):
    out = np.empty((B, T, C), np.float32)
    for c in range(NCORES):
        r = results[c]["outT"]  # [B, C, TW]
        for bb in range(B):
            out[bb, c * TW:(c + 1) * TW, :] = r[bb].T
    return out


def kernel(**inputs):
    x = np.asarray(inputs["x"], dtype=np.float32)
    Wq = np.asarray(inputs["Wq"], dtype=np.float32)
    Wk = np.asarray(inputs["Wk"], dtype=np.float32)
    Wv = np.asarray(inputs["Wv"], dtype=np.float32)
    Wo = np.asarray(inputs["Wo"], dtype=np.float32)
    nc = build_nc()
    in_maps = prepare_in_maps(x, Wq, Wk, Wv, Wo)
    res = run_bass_kernel_spmd(nc, in_maps, list(range(NCORES)))
    return assemble(res.results)


# revision 22
# speedup vs baseline: 1.5698x; 1.5698x over previous
"""Causal self-attention (B=4, T=2048, C=1024, H=16, D=64) on 8 TRN2 cores.

Sharding: core c = (batch b = c//2, head-group hg = c%2).  Each core computes
Q/K/V projections for its 8 heads over its batch, causal flash-style
attention in transposed (S^T) layout, then per-head-pair AllToAlls
redistribute the attention output yT from feature-sharded to time-sharded so
every core runs the full out-projection for one t-window of 256 across all 4
batches.  The host transposes/slices inputs and concatenates the output.

Math layout notes:
  - All matmuls run in float32r (TF32-like, 1 cycle/row for N>=256).
  - Scores are computed transposed: S^T[tk, tq] with tk on PSUM partitions,
    so the exp'd probabilities P^T feed the AV matmul directly as the moving
    operand (no P transposes).  K=64 contraction; the two heads of a pair
    write the two banks of one [128, 2, 512] PSUM tile, and a single ACT
    instruction exps both banks at once.
  - Causal masking multiplies exp'd diagonal chunks by a 0/1 mask on DVE.
  - V is stored [tk, head, 65] with a baked ones-column so a single M=65 AV
    matmul accumulates both y^T and the softmax denominator (row 64).
  - AV matmuls are emitted one chunk behind the score matmuls so the PE
    never waits on the ACT exp of the current chunk.
"""
import os
import numpy as np

import concourse.bass as bass
import concourse.mybir as mybir
import concourse.tile as tile
from concourse import bacc
from concourse.bass_utils import run_bass_kernel_spmd

B, T, C, H, D = 4, 2048, 1024, 16, 64
NCORES = 8
HPC = H // 2          # heads per core = 8
NHP = HPC // 2        # head pairs per core = 4
FPC = HPC * D         # features per core = 512
NKC = C // 128        # contraction chunks = 8
NTQ = T // 512        # tq groups of 512 = 4
NTK = T // 128        # tk chunks of 128 = 16
TW = T // NCORES      # t-window per core after A2A = 256
SCALE = 1.0 / float(np.sqrt(D))

F32 = mybir.dt.float32
F32R = mybir.dt.float32r

_CACHE = {}


def build_nc():
    if "nc" in _CACHE:
        return _CACHE["nc"]
    nc = bacc.Bacc("TRN2", target_bir_lowering=False, debug=False)

    xT_d = nc.dram_tensor("xT", [C, T], F32, kind="ExternalInput")
    wq_d = nc.dram_tensor("wqT", [C, FPC], F32, kind="ExternalInput")
    wk_d = nc.dram_tensor("wkT", [C, FPC], F32, kind="ExternalInput")
    wv_d = nc.dram_tensor("wvT", [C, FPC], F32, kind="ExternalInput")
    wo_d = nc.dram_tensor("woT", [C, C], F32, kind="ExternalInput")
    mask_d = nc.dram_tensor("masks", [128, 4, 2, 512], F32, kind="ExternalInput")
    ones_d = nc.dram_tensor("ones_in", [128, 128], F32, kind="ExternalInput")
    out_d = nc.dram_tensor("outT", [B, C, TW], F32, kind="ExternalOutput")
    debug = bool(os.environ.get("KERNEL_DEBUG"))
    if debug:
        dbg_q = nc.dram_tensor("dbg_q", [128, NHP, T], F32, kind="ExternalOutput")
        dbg_k = nc.dram_tensor("dbg_k", [128, NHP, T], F32, kind="ExternalOutput")
        dbg_v = nc.dram_tensor("dbg_v", [128, NTK, HPC, D + 1], F32, kind="ExternalOutput")
        dbg_ys = nc.dram_tensor("dbg_ys", [NHP, NCORES, 128, TW], F32, kind="ExternalOutput")
        dbg_yr = nc.dram_tensor("dbg_yr", [NHP, NCORES, 128, TW], F32, kind="ExternalOutput")
        dbg_yps = nc.dram_tensor("dbg_yps", [D + 1, 512], F32, kind="ExternalOutput")
        dbg_bca = nc.dram_tensor("dbg_bca", [D, 512], F32, kind="ExternalOutput")
        dbg_yn = nc.dram_tensor("dbg_yn", [D, 512], F32, kind="ExternalOutput")
        dbg_pt = nc.dram_tensor("dbg_pt", [128, 2, 512], F32, kind="ExternalOutput")
        dbg_rc = nc.dram_tensor("dbg_rc", [1, 512], F32, kind="ExternalOutput")

    with tile.TileContext(nc, num_cores=NCORES) as tc:
        with tc.tile_pool(name="persist", bufs=1) as persist, \
             tc.tile_pool(name="consts", bufs=1) as consts, \
             tc.tile_pool(name="dram", bufs=1, space="DRAM") as dram:

            qT = persist.tile([128, NHP, T], F32R)          # [2 heads x 64 d, hp, t]
            kT = persist.tile([128, NHP, T], F32R)
            v = persist.tile([128, NTK, HPC, D + 1], F32R)  # [tk%128, tkc, h, d+ones]
            ones_row = consts.tile([1, 64], F32)
            # per-head-pair A2A buffers: [shard(=dest core), 128 rows, 256]
            y_send = [dram.tile([NCORES * 128, TW], F32, name=f"y_send{hp}") for hp in range(NHP)]
            y_recv = [dram.tile([NCORES * 128, TW], F32, name=f"y_recv{hp}") for hp in range(NHP)]

            nc.sync.dma_start(out=ones_row, in_=ones_d[0:1, 0:64])
            # bake the ones column of v: v[:, tkc, h, 64] = 1.0
            nc.sync.dma_start(
                out=v[:, :, :, D:D + 1],
                in_=ones_d[:].rearrange("p (c h o) -> p c h o", h=HPC, o=1).bitcast(F32R),
            )

            # ---------------- Phase 1: Q/K/V projections ----------------
            with tc.tile_pool(name="weights", bufs=1) as wpool, \
                 tc.tile_pool(name="xs", bufs=2) as xpool, \
                 tc.tile_pool(name="ps1", bufs=6, space="PSUM") as ps1:
                wq = wpool.tile([128, NKC, FPC], F32R)
                wk = wpool.tile([128, NKC, FPC], F32R)
                wv = wpool.tile([128, NKC, FPC], F32R)
                # per-k-chunk DMAs so the first matmuls start early
                wq_v = wq_d[:].rearrange("(c p) f -> p c f", p=128).bitcast(F32R)
                wk_v = wk_d[:].rearrange("(c p) f -> p c f", p=128).bitcast(F32R)
                wv_v = wv_d[:].rearrange("(c p) f -> p c f", p=128).bitcast(F32R)
                xT_v = xT_d[:].rearrange("(c p) t -> p c t", p=128).bitcast(F32R)
                for kc in range(NKC):
                    nc.sync.dma_start(out=wq[:, kc, :], in_=wq_v[:, kc, :])

                for tg in range(NTQ):
                    xs = xpool.tile([128, NKC, 512], F32R)
                    for kc in range(NKC):
                        nc.sync.dma_start(out=xs[:, kc, :], in_=xT_v[:, kc, tg * 512:(tg + 1) * 512])
                    if tg == 0:
                        for kc in range(NKC):
                            nc.sync.dma_start(out=wk[:, kc, :], in_=wk_v[:, kc, :])
                            nc.sync.dma_start(out=wv[:, kc, :], in_=wv_v[:, kc, :])
                    for hp in range(NHP):
                        for which, wt, dst in (("q", wq, qT), ("k", wk, kT)):
                            ps = ps1.tile([128, 512], F32, tag="p1", name=f"ps_{which}{hp}_{tg}")
                            for kc in range(NKC):
                                nc.tensor.matmul(
                                    ps,
                                    wt[:, kc, hp * 128:(hp + 1) * 128],
                                    xs[:, kc, :],
                                    start=(kc == 0), stop=(kc == NKC - 1),
                                )
                            nc.vector.tensor_copy(dst[:, hp, tg * 512:(tg + 1) * 512], ps)
                    for sub in range(4):
                        tkc = tg * 4 + sub
                        ps = ps1.tile([128, FPC], F32, tag="p1", name=f"ps_v{tkc}")
                        for kc in range(NKC):
                            nc.tensor.matmul(
                                ps,
                                xs[:, kc, sub * 128:(sub + 1) * 128],
                                wv[:, kc, :],
                                start=(kc == 0), stop=(kc == NKC - 1),
                            )
                        nc.vector.tensor_copy(
                            v[:, tkc, :, 0:D],
                            ps[:].rearrange("p (h d) -> p h d", h=HPC),
                        )

            # ---------------- Phase 2: causal attention + per-hp A2A ----------------
            with tc.tile_pool(name="attn", bufs=1) as apool, \
                 tc.tile_pool(name="pt", bufs=4) as ptpool, \
                 tc.tile_pool(name="ynorm", bufs=2) as ypool, \
                 tc.tile_pool(name="ps_s", bufs=2, space="PSUM") as ps_s, \
                 tc.tile_pool(name="ps_y", bufs=1, space="PSUM") as ps_y, \
                 tc.tile_pool(name="ps_b", bufs=2, space="PSUM") as ps_b:
                masks = apool.tile([128, 4, 2, 512], F32R)
                nc.sync.dma_start(out=masks, in_=mask_d[:].bitcast(F32R))

                for hp in range(NHP):
                    for j in range(NTQ):
                        y_ps = [ps_y.tile([D + 1, 512], F32, tag=f"y{w}", name=f"yps{w}_{hp}_{j}")
                                for w in range(2)]
                        n_i = 4 * j + 4
                        prev = None
                        paired = not os.environ.get("KERNEL_UNPAIRED")
                        for i in range(n_i):
                            if paired:
                                s_ps = ps_s.tile([128, 2, 512], F32, tag="s", name=f"sps_{hp}_{j}_{i}")
                                s_views = [s_ps[:, w, :] for w in range(2)]
                            else:
                                s0 = ps_s.tile([128, 512], F32, tag="s0", name=f"sps0_{hp}_{j}_{i}")
                                s1 = ps_s.tile([128, 512], F32, tag="s1", name=f"sps1_{hp}_{j}_{i}")
                                s_views = [s0, s1]
                            for w in range(2):
                                nc.tensor.matmul(
                                    s_views[w],
                                    kT[w * D:(w + 1) * D, hp, i * 128:(i + 1) * 128],
                                    qT[w * D:(w + 1) * D, hp, j * 512:(j + 1) * 512],
                                    start=True, stop=True,
                                )
                            p_t = ptpool.tile([128, 2, 512], F32R, tag="p", name=f"pt_{hp}_{j}_{i}")
                            if paired:
                                nc.scalar.activation(p_t, s_ps,
                                                     mybir.ActivationFunctionType.Exp,
                                                     scale=SCALE)
                            else:
                                for w in range(2):
                                    nc.scalar.activation(p_t[:, w, :], s_views[w],
                                                         mybir.ActivationFunctionType.Exp,
                                                         scale=SCALE)
                            m = i - 4 * j
                            if m >= 0:
                                with nc.allow_low_precision(reason="exact 0/1 mask multiply in f32r"):
                                    nc.vector.tensor_mul(p_t, p_t, masks[:, m, :, :])
                            if prev is not None:
                                pp, pi = prev
                                for w in range(2):
                                    nc.tensor.matmul(y_ps[w], v[:, pi, 2 * hp + w, :], pp[:, w, :],
                                                     start=(pi == 0), stop=False)
                            if debug and hp == 0 and j == 0 and i == 1:
                                nc.sync.dma_start(out=dbg_pt[:], in_=p_t[:].bitcast(F32))
                            prev = (p_t, i)
                        pp, pi = prev
                        for w in range(2):
                            nc.tensor.matmul(y_ps[w], v[:, pi, 2 * hp + w, :], pp[:, w, :],
                                             start=(pi == 0), stop=True)
                        for w in range(2):
                            se_sb = ypool.tile([1, 512], F32, tag=f"se{w}", name=f"se{w}_{hp}_{j}")
                            nc.vector.tensor_copy(se_sb, y_ps[w][D:D + 1, :])
                            recip = ypool.tile([1, 512], F32, tag=f"r{w}", name=f"rc{w}_{hp}_{j}")
                            nc.vector.reciprocal_approx_fast(recip, se_sb)
                            y_sb = ypool.tile([D, 512], F32, tag=f"ysb{w}", name=f"ysb{w}_{hp}_{j}")
                            nc.vector.tensor_copy(y_sb, y_ps[w][0:D, :])
                            bca = ps_b.tile([D, 512], F32, tag="b", name=f"bca{w}_{hp}_{j}")
                            nc.tensor.matmul(bca, ones_row, recip, start=True, stop=True)
                            yn = ypool.tile([D, 512], F32, tag=f"yn{w}", name=f"yn{w}_{hp}_{j}")
                            nc.vector.tensor_mul(yn, y_sb, bca)
                            if debug and hp == 0 and j == 0 and w == 0:
                                dt1 = ypool.tile([D + 1, 512], F32, tag="dbg1", name="dbg_t1")
                                nc.vector.tensor_copy(dt1, y_ps[w])
                                nc.sync.dma_start(out=dbg_yps[:], in_=dt1)
                                dt2 = ypool.tile([D, 512], F32, tag="dbg2", name="dbg_t2")
                                nc.vector.tensor_copy(dt2, bca)
                                nc.sync.dma_start(out=dbg_bca[:], in_=dt2)
                                nc.sync.dma_start(out=dbg_yn[:], in_=yn)
                                nc.sync.dma_start(out=dbg_rc[:], in_=recip)
                            # rows (w*64 .. +64) of this hp's 128-row block,
                            # split across the two 256-wide A2A shards 2j, 2j+1
                            for s in range(2):
                                nc.sync.dma_start(
                                    out=y_send[hp][:]
                                    .rearrange("(s r) t -> s r t", r=128)
                                    [2 * j + s, w * D:(w + 1) * D, :],
                                    in_=yn[:, s * TW:(s + 1) * TW],
                                )
                    # fire this head-pair's A2A; overlaps with the next pair's attention
                    nc.gpsimd.collective_compute(
                        "AllToAll",
                        mybir.AluOpType.bypass,
                        replica_groups=[list(range(NCORES))],
                        ins=[y_send[hp].opt()],
                        outs=[y_recv[hp].opt()],
                    )

            if debug:
                nc.sync.dma_start(out=dbg_q[:], in_=qT[:].bitcast(F32))
                nc.sync.dma_start(out=dbg_k[:], in_=kT[:].bitcast(F32))
                nc.sync.dma_start(out=dbg_v[:], in_=v[:].bitcast(F32))
                for hp in range(NHP):
                    nc.sync.dma_start(
                        out=dbg_ys[hp],
                        in_=y_send[hp][:].rearrange("(s r) t -> s r t", r=128))
                    nc.sync.dma_start(
                        out=dbg_yr[hp],
                        in_=y_recv[hp][:].rearrange("(s r) t -> s r t", r=128))

            # ---------------- Phase 4: out-projection ----------------
            tc.strict_bb_all_engine_barrier()
            with tc.tile_pool(name="wo", bufs=1) as wopool, \
                 tc.tile_pool(name="yb", bufs=2) as ybpool, \
                 tc.tile_pool(name="oev", bufs=3) as oevpool, \
                 tc.tile_pool(name="ps4", bufs=6, space="PSUM") as ps4:
                wo = wopool.tile([128, NKC, C], F32R)
                wo_v = wo_d[:].rearrange("(c p) o -> p c o", p=128).bitcast(F32R)
                for kc in range(NKC):
                    nc.sync.dma_start(out=wo[:, kc, :], in_=wo_v[:, kc, :])
                for bb in range(B):
                    yb = ybpool.tile([128, NKC, TW], F32R)
                    # f-chunk fc = (rank-half fc//4, head-pair fc%4)
                    for fc in range(NKC):
                        rk = 2 * bb + fc // 4
                        nc.sync.dma_start(
                            out=yb[:, fc, :],
                            in_=y_recv[fc % 4][:]
                            .rearrange("(s r) t -> s r t", r=128)[rk, :, :]
                            .bitcast(F32R),
                        )
                    for oc in range(NKC):
                        ps = ps4.tile([128, TW], F32, tag="p4", name=f"ps4_{bb}_{oc}")
                        for fc in range(NKC):
                            nc.tensor.matmul(
                                ps,
                                wo[:, fc, oc * 128:(oc + 1) * 128],
                                yb[:, fc, :],
                                start=(fc == 0), stop=(fc == NKC - 1),
                            )
                        oe = oevpool.tile([128, TW], F32, tag="oe", name=f"oe_{bb}_{oc}")
                        nc.vector.tensor_copy(oe, ps)
                        nc.sync.dma_start(out=out_d[bb, oc * 128:(oc + 1) * 128, :], in_=oe)

    nc.compile()
    _CACHE["nc"] = nc
    return nc


def host_constants():
    # multiplicative 0/1 masks for diagonal chunks: variant m is used for
    # tk-chunk i = 4j+m against tq-window j; allowed iff q >= p + 128m
    masks = np.zeros((128, 4, 2, 512), np.float32)
    q_idx = np.arange(512)[None, :]
    p_idx = np.arange(128)[:, None]
    for m in range(4):
        mm = (q_idx >= p_idx + 128 * m).astype(np.float32)
        masks[:, m, 0, :] = mm
        masks[:, m, 1, :] = mm
    ones_in = np.ones((128, 128), np.float32)
    return masks, ones_in


def prepare_in_maps(x, Wq, Wk, Wv, Wo):
    masks, ones_in = host_constants()
    woT = np.ascontiguousarray(Wo.T.astype(np.float32, copy=False))
    wqT_f = np.ascontiguousarray(Wq.T.astype(np.float32, copy=False))
    wkT_f = np.ascontiguousarray(Wk.T.astype(np.float32, copy=False))
    wvT_f = np.ascontiguousarray(Wv.T.astype(np.float32, copy=False))
    in_maps = []
    for c in range(NCORES):
        b, hg = c // 2, c % 2
        fs = slice(hg * FPC, (hg + 1) * FPC)
        in_maps.append({
            "xT": np.ascontiguousarray(x[b].T),
            "wqT": np.ascontiguousarray(wqT_f[:, fs]),
            "wkT": np.ascontiguousarray(wkT_f[:, fs]),
            "wvT": np.ascontiguousarray(wvT_f[:, fs]),
            "woT": woT,
            "masks": masks,
            "ones_in": ones_in,
        })
    return in_maps


def assemble(results):
    out = np.empty((B, T, C), np.float32)
    for c in range(NCORES):
        r = results[c]["outT"]  # [B, C, TW]
        for bb in range(B):
            out[bb, c * TW:(c + 1) * TW, :] = r[bb].T
    return out


def kernel(**inputs):
    x = np.asarray(inputs["x"], dtype=np.float32)
    Wq = np.asarray(inputs["Wq"], dtype=np.float32)
    Wk = np.asarray(inputs["Wk"], dtype=np.float32)
    Wv = np.asarray(inputs["Wv"], dtype=np.float32)
    Wo = np.asarray(inputs["Wo"], dtype=np.float32)
    nc = build_nc()
    in_maps = prepare_in_maps(x, Wq, Wk, Wv, Wo)
    res = run_bass_kernel_spmd(nc, in_maps, list(range(NCORES)))
    return assemble(res.results)
